# revision 22
# baseline (speedup 1.0000x reference)
"""DTCWT Qshift layer (level-2) Bass kernel for 8 Trainium2 NeuronCores.

Math: all four subband images are separable 2-D transforms of each input
image X (256x256, per (batch, channel)):

    LoLo   = C_lo  @ X @ R_lo^T
    LoHi   = C_hi  @ X @ R_lo^T   -> q2c -> bands 15/165
    HiLo   = C_lo  @ X @ R_hi^T   -> q2c -> bands 75/105
    MidMid = C_mid @ X @ R_mid^T  -> q2c -> bands 45/135

where C_f == R_f are 128x256 decimating filter matrices built from the
qshift filter taps (including the dtcwt symmetric-extension boundary
handling and the two-tree odd/even phase interleave).

On-device plan (per image, fp32):
  col pass:  T_f^T[c, i] via matmul(lhsT=X_chunk, rhs=CT3_chunk) accumulated
             over two 128-row chunks -- the stationary operand is read
             transposed, so the intermediate lands with c on partitions and
             no explicit transpose is ever needed.
  row pass:  matmul(lhsT=T_f^T chunk, rhs=R_pack chunk) accumulated over the
             two c chunks, producing final (i, j) layout directly in PSUM.
  q2c:       the C-matrix columns are permuted to [even rows | odd rows] and
             the band R-matrices to [even cols | odd cols], so the a/b/c/d
             quad slices are contiguous 64x64 blocks and each of Yhr/Yhi is
             two strided tensor_add/sub ops.

Sharding: pure data parallel, 2 batches (128 images) per core, filters
replicated. No collectives.
"""

import contextlib

import numpy as np

import concourse.bacc as bacc
import concourse.mybir as mybir
from concourse.tile import TileContext
from concourse.bass_utils import run_bass_kernel_spmd

N_CORES = 8
B, C, H, W = 16, 64, 256, 256
M = 14
N_IMG = (B * C) // N_CORES  # images per core
F32 = mybir.dt.float32
F32R = mybir.dt.float32r

# float32r streams matmuls at 1 cycle/row (vs 4 for exact float32) at a
# small mantissa cost; flip here if precision turns out marginal.
USE_F32R = False


# ----------------------------------------------------------------------------
# Host-side: build the 128x256 decimating filter matrices (tiny, numpy).
# ----------------------------------------------------------------------------

def _reflect(x, minx, maxx):
    x = np.asarray(x, dtype=np.float64)
    rng = maxx - minx
    mod = np.fmod(x - minx, 2.0 * rng)
    mod = np.where(mod < 0.0, mod + 2.0 * rng, mod)
    out = np.where(mod >= rng, 2.0 * rng - mod, mod) + minx
    return np.round(out).astype(np.int64)


def _conv_mat(l, ha, hb, highpass):
    """Matrix D (l//2, l) with out = D @ x matching coldfilt/rowdfilt."""
    xe = _reflect(np.arange(-M, l + M), -0.5, l - 0.5)
    m1 = np.zeros((l // 4, l), np.float32)
    m2 = np.zeros((l // 4, l), np.float32)
    for k in range(l // 4):
        for m in range(M):
            m1[k, xe[2 + 4 * k + 2 * m]] += ha[m]
            m2[k, xe[3 + 4 * k + 2 * m]] += hb[m]
    if highpass:
        m1, m2 = m2, m1
    out = np.zeros((l // 2, l), np.float32)
    out[0::2] = m1
    out[1::2] = m2
    return out


def _build_consts(h0a, h0b, h1a, h1b, h2a, h2b):
    sc = np.float32(np.sqrt(0.5))
    c_lo = _conv_mat(H, h0b, h0a, False)
    c_hi = _conv_mat(H, h1b, h1a, True)
    c_mid = _conv_mat(H, h2b, h2a, True)

    iperm = np.concatenate([np.arange(0, 128, 2), np.arange(1, 128, 2)])

    # CT3[rc] : [128, 384] = [C_lo^T | C_hi^T | C_mid^T] chunk rc, rows
    # permuted to [even | odd].
    ct3 = np.zeros((2, 128, 384), np.float32)
    for fi, cf in enumerate((c_lo, c_hi, c_mid)):
        cp = cf[iperm]  # (128, 256)
        for rc in range(2):
            ct3[rc, :, fi * 128:(fi + 1) * 128] = cp[:, rc * 128:(rc + 1) * 128].T

    # Row-pass rhs packs (chunked over c):
    #   RT_BC = [sc*R_lo (jperm) | sc*R_mid (jperm)]
    #   RT_A  = [sc*R_hi (jperm) | R_lo (natural j)]
    r_lo_s = (sc * c_lo)[iperm]
    r_mid_s = (sc * c_mid)[iperm]
    r_hi_s = (sc * c_hi)[iperm]
    rt_bc = np.zeros((2, 128, 256), np.float32)
    rt_a = np.zeros((2, 128, 256), np.float32)
    for cc in range(2):
        sl = slice(cc * 128, (cc + 1) * 128)
        rt_bc[cc, :, 0:128] = r_lo_s[:, sl].T
        rt_bc[cc, :, 128:256] = r_mid_s[:, sl].T
        rt_a[cc, :, 0:128] = r_hi_s[:, sl].T
        rt_a[cc, :, 128:256] = c_lo[:, sl].T  # natural row order for LoLo
    return ct3, rt_bc, rt_a


# ----------------------------------------------------------------------------
# Device kernel
# ----------------------------------------------------------------------------

def _build_nc(n_img, f32r=USE_F32R, repeat=1):
    """repeat>1 wraps the whole body in a hardware loop — benchmarking only
    (same inputs reprocessed; amortizes the host dispatch overhead)."""
    mm = lambda ap: ap  # matmul operand dtype comes from the tiles below
    fin = F32R if f32r else F32  # dtype for matmul-feeding tensors
    nc = bacc.Bacc("TRN2", target_bir_lowering=False)

    x_d = nc.dram_tensor("X", (n_img, H, W), fin, kind="ExternalInput")
    ct3_d = nc.dram_tensor("CT3", (2, 128, 384), fin, kind="ExternalInput")
    rtbc_d = nc.dram_tensor("RT_BC", (2, 128, 256), fin, kind="ExternalInput")
    rta_d = nc.dram_tensor("RT_A", (2, 128, 256), fin, kind="ExternalInput")

    lolo_d = nc.dram_tensor("LoLo", (n_img, 128, 128), F32, kind="ExternalOutput")
    yhr_d = nc.dram_tensor("Yhr", (n_img, 6, 64, 64), F32, kind="ExternalOutput")
    yhi_d = nc.dram_tensor("Yhi", (n_img, 6, 64, 64), F32, kind="ExternalOutput")

    with TileContext(nc) as tc:
        with (
            tc.tile_pool(name="consts", bufs=1) as cpool,
            tc.tile_pool(name="xin", bufs=3) as xpool,
            tc.tile_pool(name="mid", bufs=2) as mpool,
            tc.tile_pool(name="outs", bufs=3) as opool,
            tc.tile_pool(name="ps_t1t", bufs=1, space="PSUM") as pst,
            tc.tile_pool(name="ps_row", bufs=2, space="PSUM") as psr,
        ):
            ct3_sb = cpool.tile([128, 2, 384], fin)
            nc.sync.dma_start(out=ct3_sb, in_=ct3_d.rearrange("rc p n -> p rc n"))
            rtbc_sb = cpool.tile([128, 2, 256], fin)
            nc.sync.dma_start(out=rtbc_sb, in_=rtbc_d.rearrange("rc p n -> p rc n"))
            rta_sb = cpool.tile([128, 2, 256], fin)
            nc.sync.dma_start(out=rta_sb, in_=rta_d.rearrange("rc p n -> p rc n"))

            rep = tc.For_i(0, repeat, 1) if repeat > 1 else contextlib.nullcontext()
            with rep:
              for n in range(n_img):
                # ---- load image (256x256) as [128, rc, 256]
                x_sb = xpool.tile([128, 2, 256], fin)
                nc.sync.dma_start(
                    out=x_sb, in_=x_d[n].rearrange("(rc p) c -> p rc c", rc=2)
                )

                # ---- column pass: T^T[c, (f, k)] in PSUM  (2 c-chunks)
                t1t_ps = pst.tile([128, 1024], F32)  # cc0 -> [0:384], cc1 -> [512:896]
                for cc in range(2):
                    for rc in range(2):
                        nc.tensor.matmul(
                            t1t_ps[:, cc * 512:cc * 512 + 384],
                            lhsT=mm(x_sb[:, rc, cc * 128:(cc + 1) * 128]),
                            rhs=mm(ct3_sb[:, rc, :]),
                            start=(rc == 0),
                            stop=(rc == 1),
                        )

                t1ta_sb = mpool.tile([128, 384], fin)
                t1tb_sb = mpool.tile([128, 384], fin)
                nc.vector.tensor_copy(out=t1ta_sb, in_=t1t_ps[:, 0:384])
                nc.scalar.copy(out=t1tb_sb, in_=t1t_ps[:, 512:896])

                # ---- row pass (accumulate over the two c chunks)
                out_b = psr.tile([128, 256], F32)  # [sc*LoHi | junk]
                out_c = psr.tile([128, 256], F32)  # [junk | sc*MidMid]
                out_a = psr.tile([128, 256], F32)  # [sc*HiLo | LoLo]
                for cc, t1t_sb in enumerate((t1ta_sb, t1tb_sb)):
                    st, sp = (cc == 0), (cc == 1)
                    nc.tensor.matmul(
                        out_b, lhsT=mm(t1t_sb[:, 128:256]),
                        rhs=mm(rtbc_sb[:, cc, :]), start=st, stop=sp,
                    )
                    nc.tensor.matmul(
                        out_c, lhsT=mm(t1t_sb[:, 256:384]),
                        rhs=mm(rtbc_sb[:, cc, :]), start=st, stop=sp,
                    )
                    nc.tensor.matmul(
                        out_a, lhsT=mm(t1t_sb[:, 0:128]),
                        rhs=mm(rta_sb[:, cc, :]), start=st, stop=sp,
                    )

                # ---- stage bands to SBUF: [LoHi | MidMid | HiLo], LoLo
                band_sb = opool.tile([128, 3, 2, 64], F32)
                nc.vector.tensor_copy(
                    out=band_sb[:, 0], in_=out_b[:, 0:128].rearrange("p (h j) -> p h j", h=2)
                )
                nc.vector.tensor_copy(
                    out=band_sb[:, 1], in_=out_c[:, 128:256].rearrange("p (h j) -> p h j", h=2)
                )
                nc.vector.tensor_copy(
                    out=band_sb[:, 2], in_=out_a[:, 0:128].rearrange("p (h j) -> p h j", h=2)
                )
                lolo_sb = opool.tile([128, 128], F32)
                nc.scalar.copy(out=lolo_sb, in_=out_a[:, 128:256])

                # ---- q2c: a,b,c,d are contiguous 64x64 blocks.
                # TensorTensor requires equal base partitions, so shift the
                # odd-row half (c/d) down to partitions 0:64 via SBUF DMA.
                cd_sb = opool.tile([64, 3, 2, 64], F32)
                nc.sync.dma_start(out=cd_sb, in_=band_sb[64:128])
                yhr_sb = opool.tile([64, 6, 64], F32)
                yhi_sb = opool.tile([64, 6, 64], F32)
                a_f = band_sb[0:64, :, 0, :]       # [64, 3, 64]
                b_f = band_sb[0:64, :, 1, :]
                c_f = cd_sb[:, :, 0, :]
                d_f = cd_sb[:, :, 1, :]
                a_r = band_sb[0:64, 2::-1, 0, :]   # reversed band order
                b_r = band_sb[0:64, 2::-1, 1, :]
                c_r = cd_sb[:, 2::-1, 0, :]
                d_r = cd_sb[:, 2::-1, 1, :]
                nc.vector.tensor_sub(out=yhr_sb[:, 0:3], in0=a_f, in1=d_f)
                nc.vector.tensor_add(out=yhr_sb[:, 3:6], in0=a_r, in1=d_r)
                nc.gpsimd.tensor_add(out=yhi_sb[:, 0:3], in0=b_f, in1=c_f)
                nc.gpsimd.tensor_sub(out=yhi_sb[:, 3:6], in0=b_r, in1=c_r)

                # ---- store
                nc.sync.dma_start(out=lolo_d[n][0::2], in_=lolo_sb[0:64])
                nc.sync.dma_start(out=lolo_d[n][1::2], in_=lolo_sb[64:128])
                nc.sync.dma_start(
                    out=yhr_d[n].rearrange("b i j -> i b j"), in_=yhr_sb
                )
                nc.sync.dma_start(
                    out=yhi_d[n].rearrange("b i j -> i b j"), in_=yhi_sb
                )
    nc.finalize()
    return nc


_NC_CACHE = {}

# Set TRACE=True (e.g. from test.py) to capture an NTFF profile; the
# BassKernelResults lands in LAST_RESULT.
TRACE = False
LAST_RESULT = None


def _get_nc(n_img):
    if n_img not in _NC_CACHE:
        _NC_CACHE[n_img] = _build_nc(n_img)
    return _NC_CACHE[n_img]


def kernel(X, h0a, h0b, h1a, h1b, h2a, h2b):
    X = np.asarray(X, np.float32)
    ct3, rt_bc, rt_a = _build_consts(
        np.asarray(h0a, np.float32), np.asarray(h0b, np.float32),
        np.asarray(h1a, np.float32), np.asarray(h1b, np.float32),
        np.asarray(h2a, np.float32), np.asarray(h2b, np.float32),
    )
    nc = _get_nc(N_IMG)
    bpc = B // N_CORES  # batches per core
    in_maps = []
    for core in range(N_CORES):
        xs = np.ascontiguousarray(
            X[core * bpc:(core + 1) * bpc].reshape(N_IMG, H, W)
        )
        in_maps.append({"X": xs, "CT3": ct3, "RT_BC": rt_bc, "RT_A": rt_a})
    global LAST_RESULT
    res = run_bass_kernel_spmd(
        nc, in_maps, core_ids=list(range(N_CORES)), trace=TRACE
    )
    LAST_RESULT = res
    lolo = np.concatenate(
        [r["LoLo"].reshape(bpc, C, 128, 128) for r in res.results]
    )
    yhr = np.concatenate(
        [r["Yhr"].reshape(bpc, C, 6, 64, 64) for r in res.results]
    )
    yhi = np.concatenate(
        [r["Yhi"].reshape(bpc, C, 6, 64, 64) for r in res.results]
    )
    return lolo, yhr, yhi


# revision 26
# speedup vs baseline: 1.4222x; 1.4222x over previous
"""DTCWT Qshift layer (level-2) Bass kernel for 8 Trainium2 NeuronCores.

Math: all four subband images are separable 2-D transforms of each input
image X (256x256, per (batch, channel)):

    LoLo   = C_lo  @ X @ R_lo^T
    LoHi   = C_hi  @ X @ R_lo^T   -> q2c -> bands 15/165
    HiLo   = C_lo  @ X @ R_hi^T   -> q2c -> bands 75/105
    MidMid = C_mid @ X @ R_mid^T  -> q2c -> bands 45/135

where C_f == R_f are 128x256 decimating filter matrices built from the
qshift filter taps (including the dtcwt symmetric-extension boundary
handling and the two-tree odd/even phase interleave).

On-device plan (per image, fp32):
  col pass:  T_f^T[c, i] via matmul(lhsT=X_chunk, rhs=CT3_chunk) accumulated
             over two 128-row chunks -- the stationary operand is read
             transposed, so the intermediate lands with c on partitions and
             no explicit transpose is ever needed.
  row pass:  matmul(lhsT=T_f^T chunk, rhs=R_pack chunk) accumulated over the
             two c chunks, producing final (i, j) layout directly in PSUM.
  q2c:       the C-matrix columns are permuted to [even rows | odd rows] and
             the band R-matrices to [even cols | odd cols], so the a/b/c/d
             quad slices are contiguous 64x64 blocks and each of Yhr/Yhi is
             two strided tensor_add/sub ops.

Sharding: pure data parallel, 2 batches (128 images) per core, filters
replicated. No collectives.
"""

import contextlib

import numpy as np

import concourse.bacc as bacc
import concourse.mybir as mybir
from concourse.tile import TileContext
from concourse.bass_utils import run_bass_kernel_spmd

N_CORES = 8
B, C, H, W = 16, 64, 256, 256
M = 14
N_IMG = (B * C) // N_CORES  # images per core
F32 = mybir.dt.float32
F32R = mybir.dt.float32r

# float32r streams matmuls at 1 cycle/row (vs 4 for exact float32) at a
# small mantissa cost; flip here if precision turns out marginal.
USE_F32R = False


# ----------------------------------------------------------------------------
# Host-side: build the 128x256 decimating filter matrices (tiny, numpy).
# ----------------------------------------------------------------------------

def _reflect(x, minx, maxx):
    x = np.asarray(x, dtype=np.float64)
    rng = maxx - minx
    mod = np.fmod(x - minx, 2.0 * rng)
    mod = np.where(mod < 0.0, mod + 2.0 * rng, mod)
    out = np.where(mod >= rng, 2.0 * rng - mod, mod) + minx
    return np.round(out).astype(np.int64)


def _conv_mat(l, ha, hb, highpass):
    """Matrix D (l//2, l) with out = D @ x matching coldfilt/rowdfilt."""
    xe = _reflect(np.arange(-M, l + M), -0.5, l - 0.5)
    m1 = np.zeros((l // 4, l), np.float32)
    m2 = np.zeros((l // 4, l), np.float32)
    for k in range(l // 4):
        for m in range(M):
            m1[k, xe[2 + 4 * k + 2 * m]] += ha[m]
            m2[k, xe[3 + 4 * k + 2 * m]] += hb[m]
    if highpass:
        m1, m2 = m2, m1
    out = np.zeros((l // 2, l), np.float32)
    out[0::2] = m1
    out[1::2] = m2
    return out


def _build_consts(h0a, h0b, h1a, h1b, h2a, h2b):
    sc = np.float32(np.sqrt(0.5))
    c_lo = _conv_mat(H, h0b, h0a, False)
    c_hi = _conv_mat(H, h1b, h1a, True)
    c_mid = _conv_mat(H, h2b, h2a, True)

    iperm = np.concatenate([np.arange(0, 128, 2), np.arange(1, 128, 2)])

    # CT3[rc] : [128, 384] = [C_lo^T | C_hi^T | C_mid^T] chunk rc, rows
    # permuted to [even | odd].
    ct3 = np.zeros((2, 128, 384), np.float32)
    for fi, cf in enumerate((c_lo, c_hi, c_mid)):
        cp = cf[iperm]  # (128, 256)
        for rc in range(2):
            ct3[rc, :, fi * 128:(fi + 1) * 128] = cp[:, rc * 128:(rc + 1) * 128].T

    # Row-pass rhs packs (chunked over c):
    #   RT_BC = [sc*R_lo (jperm) | sc*R_mid (jperm)]
    #   RT_A  = [sc*R_hi (jperm) | R_lo (natural j)]
    r_lo_s = (sc * c_lo)[iperm]
    r_mid_s = (sc * c_mid)[iperm]
    r_hi_s = (sc * c_hi)[iperm]
    rt_bc = np.zeros((2, 128, 256), np.float32)
    rt_a = np.zeros((2, 128, 256), np.float32)
    for cc in range(2):
        sl = slice(cc * 128, (cc + 1) * 128)
        rt_bc[cc, :, 0:128] = r_lo_s[:, sl].T
        rt_bc[cc, :, 128:256] = r_mid_s[:, sl].T
        rt_a[cc, :, 0:128] = r_hi_s[:, sl].T
        rt_a[cc, :, 128:256] = c_lo[:, sl].T  # natural row order for LoLo
    return ct3, rt_bc, rt_a


# ----------------------------------------------------------------------------
# Device kernel
# ----------------------------------------------------------------------------

def _build_nc(n_img, f32r=USE_F32R, repeat=1):
    """repeat>1 wraps the whole body in a hardware loop — benchmarking only
    (same inputs reprocessed; amortizes the host dispatch overhead)."""
    mm = lambda ap: ap  # matmul operand dtype comes from the tiles below
    fin = F32R if f32r else F32  # dtype for matmul-feeding tensors
    nc = bacc.Bacc("TRN2", target_bir_lowering=False)

    x_d = nc.dram_tensor("X", (n_img, H, W), fin, kind="ExternalInput")
    ct3_d = nc.dram_tensor("CT3", (2, 128, 384), fin, kind="ExternalInput")
    rtbc_d = nc.dram_tensor("RT_BC", (2, 128, 256), fin, kind="ExternalInput")
    rta_d = nc.dram_tensor("RT_A", (2, 128, 256), fin, kind="ExternalInput")

    lolo_d = nc.dram_tensor("LoLo", (n_img, 128, 128), F32, kind="ExternalOutput")
    yhr_d = nc.dram_tensor("Yhr", (n_img, 6, 64, 64), F32, kind="ExternalOutput")
    yhi_d = nc.dram_tensor("Yhi", (n_img, 6, 64, 64), F32, kind="ExternalOutput")

    with TileContext(nc) as tc:
        with (
            tc.tile_pool(name="consts", bufs=1) as cpool,
            tc.tile_pool(name="xin", bufs=2) as xpool,
            tc.tile_pool(name="mid", bufs=2) as mpool,
            tc.tile_pool(name="outs", bufs=2) as opool,
            tc.tile_pool(name="ps_t1t", bufs=2, space="PSUM") as pst,
            tc.tile_pool(name="ps_row", bufs=2, space="PSUM") as psr,
        ):
            ct3_sb = cpool.tile([128, 2, 384], fin)
            nc.sync.dma_start(out=ct3_sb, in_=ct3_d.rearrange("rc p n -> p rc n"))
            rtbc_sb = cpool.tile([128, 2, 256], fin)
            nc.sync.dma_start(out=rtbc_sb, in_=rtbc_d.rearrange("rc p n -> p rc n"))
            rta_sb = cpool.tile([128, 2, 256], fin)
            nc.sync.dma_start(out=rta_sb, in_=rta_d.rearrange("rc p n -> p rc n"))

            G = 8  # images per DMA group (HWDGE descriptor-gen is a flat
            #        ~625ns per dma_start, so batch transfers across images)
            assert n_img % G == 0
            rep = tc.For_i(0, repeat, 1) if repeat > 1 else contextlib.nullcontext()
            with rep:
              for g0 in range(0, n_img, G):
                # ---- load G images (256x256 each) as [128, g, rc, 256]
                x_sb = xpool.tile([128, G, 2, 256], fin)
                nc.sync.dma_start(
                    out=x_sb,
                    in_=x_d[g0:g0 + G].rearrange("g (rc p) c -> p g rc c", rc=2),
                )

                band_sb = opool.tile([128, G, 3, 2, 64], F32)
                lolo_sb = opool.tile([128, G, 128], F32)

                for k in range(G):
                    # ---- column pass: T^T[c, (f, i)] in PSUM (2 c-chunks)
                    t1t_ps = pst.tile([128, 1024], F32)
                    for cc in range(2):
                        for rc in range(2):
                            nc.tensor.matmul(
                                t1t_ps[:, cc * 512:cc * 512 + 384],
                                lhsT=mm(x_sb[:, k, rc, cc * 128:(cc + 1) * 128]),
                                rhs=mm(ct3_sb[:, rc, :]),
                                start=(rc == 0),
                                stop=(rc == 1),
                            )

                    t1ta_sb = mpool.tile([128, 384], fin)
                    t1tb_sb = mpool.tile([128, 384], fin)
                    nc.vector.tensor_copy(out=t1ta_sb, in_=t1t_ps[:, 0:384])
                    nc.scalar.copy(out=t1tb_sb, in_=t1t_ps[:, 512:896])

                    # ---- row pass (accumulate over the two c chunks)
                    # one 2-bank tile: B=[0:256] C=[256:512] A=[512:768]
                    row_ps = psr.tile([128, 768], F32)
                    for cc, t1t_sb in enumerate((t1ta_sb, t1tb_sb)):
                        st, sp = (cc == 0), (cc == 1)
                        nc.tensor.matmul(
                            row_ps[:, 0:256], lhsT=mm(t1t_sb[:, 128:256]),
                            rhs=mm(rtbc_sb[:, cc, :]), start=st, stop=sp,
                        )
                        nc.tensor.matmul(
                            row_ps[:, 256:512], lhsT=mm(t1t_sb[:, 256:384]),
                            rhs=mm(rtbc_sb[:, cc, :]), start=st, stop=sp,
                        )
                        nc.tensor.matmul(
                            row_ps[:, 512:768], lhsT=mm(t1t_sb[:, 0:128]),
                            rhs=mm(rta_sb[:, cc, :]), start=st, stop=sp,
                        )

                    # ---- stage bands: [LoHi | MidMid | HiLo], LoLo
                    nc.vector.tensor_copy(
                        out=band_sb[:, k, 0],
                        in_=row_ps[:, 0:128].rearrange("p (h j) -> p h j", h=2),
                    )
                    nc.vector.tensor_copy(
                        out=band_sb[:, k, 1],
                        in_=row_ps[:, 384:512].rearrange("p (h j) -> p h j", h=2),
                    )
                    nc.vector.tensor_copy(
                        out=band_sb[:, k, 2],
                        in_=row_ps[:, 512:640].rearrange("p (h j) -> p h j", h=2),
                    )
                    nc.scalar.copy(out=lolo_sb[:, k], in_=row_ps[:, 640:768])

                # ---- q2c. TensorTensor needs equal base partitions, so shift
                # the odd-row half (c/d) to partitions 0:64 with one SBUF DMA
                # for the whole group.
                cd_sb = opool.tile([64, G, 3, 2, 64], F32)
                nc.sync.dma_start(out=cd_sb, in_=band_sb[64:128])
                yhr_sb = opool.tile([64, G, 6, 64], F32)
                yhi_sb = opool.tile([64, G, 6, 64], F32)
                for k in range(G):
                    a_f = band_sb[0:64, k, :, 0, :]       # [64, 3, 64]
                    b_f = band_sb[0:64, k, :, 1, :]
                    c_f = cd_sb[:, k, :, 0, :]
                    d_f = cd_sb[:, k, :, 1, :]
                    a_r = band_sb[0:64, k, 2::-1, 0, :]   # reversed band order
                    b_r = band_sb[0:64, k, 2::-1, 1, :]
                    c_r = cd_sb[:, k, 2::-1, 0, :]
                    d_r = cd_sb[:, k, 2::-1, 1, :]
                    nc.vector.tensor_sub(out=yhr_sb[:, k, 0:3], in0=a_f, in1=d_f)
                    nc.vector.tensor_add(out=yhr_sb[:, k, 3:6], in0=a_r, in1=d_r)
                    nc.gpsimd.tensor_add(out=yhi_sb[:, k, 0:3], in0=b_f, in1=c_f)
                    nc.gpsimd.tensor_sub(out=yhi_sb[:, k, 3:6], in0=b_r, in1=c_r)

                # ---- store (one DMA per tensor per group)
                nc.sync.dma_start(
                    out=lolo_d[g0:g0 + G, 0::2, :].rearrange("g i j -> i g j"),
                    in_=lolo_sb[0:64],
                )
                nc.sync.dma_start(
                    out=lolo_d[g0:g0 + G, 1::2, :].rearrange("g i j -> i g j"),
                    in_=lolo_sb[64:128],
                )
                nc.sync.dma_start(
                    out=yhr_d[g0:g0 + G].rearrange("g b i j -> i g b j"), in_=yhr_sb
                )
                nc.sync.dma_start(
                    out=yhi_d[g0:g0 + G].rearrange("g b i j -> i g b j"), in_=yhi_sb
                )
    nc.finalize()
    return nc


_NC_CACHE = {}

# Set TRACE=True (e.g. from test.py) to capture an NTFF profile; the
# BassKernelResults lands in LAST_RESULT.
TRACE = False
LAST_RESULT = None


def _get_nc(n_img):
    if n_img not in _NC_CACHE:
        _NC_CACHE[n_img] = _build_nc(n_img)
    return _NC_CACHE[n_img]


def kernel(X, h0a, h0b, h1a, h1b, h2a, h2b):
    X = np.asarray(X, np.float32)
    ct3, rt_bc, rt_a = _build_consts(
        np.asarray(h0a, np.float32), np.asarray(h0b, np.float32),
        np.asarray(h1a, np.float32), np.asarray(h1b, np.float32),
        np.asarray(h2a, np.float32), np.asarray(h2b, np.float32),
    )
    nc = _get_nc(N_IMG)
    bpc = B // N_CORES  # batches per core
    in_maps = []
    for core in range(N_CORES):
        xs = np.ascontiguousarray(
            X[core * bpc:(core + 1) * bpc].reshape(N_IMG, H, W)
        )
        in_maps.append({"X": xs, "CT3": ct3, "RT_BC": rt_bc, "RT_A": rt_a})
    global LAST_RESULT
    res = run_bass_kernel_spmd(
        nc, in_maps, core_ids=list(range(N_CORES)), trace=TRACE
    )
    LAST_RESULT = res
    lolo = np.concatenate(
        [r["LoLo"].reshape(bpc, C, 128, 128) for r in res.results]
    )
    yhr = np.concatenate(
        [r["Yhr"].reshape(bpc, C, 6, 64, 64) for r in res.results]
    )
    yhi = np.concatenate(
        [r["Yhi"].reshape(bpc, C, 6, 64, 64) for r in res.results]
    )
    return lolo, yhr, yhi
